# revision 1
# baseline (speedup 1.0000x reference)
"""GCN (2-layer) Trainium2 kernel, 8-core SPMD.

Strategy: partition destination nodes across 8 cores (contiguous shards of
12544). Both GCNConv layers are computed as aggregate-then-transform:
    out = dinv * ( (sum_{src in N(dst)} table[src]) @ W + sqrt(deg)*b )
with table = dinv-prescaled features (so the symmetric deg^-1/2 norm is exact).
Per 128-dst window the aggregation is a PE segment-matmul:
    aggT[feat, dst] += M[msgs, feat].T @ S[msgs, dst]
where M is a dma_gather of source rows (bf16, 256B each) and S is a one-hot
built on DVE via is_equal(iota, dst_rel). h (post-relu, bf16) is exchanged
between layers with a single AllGather. The same edge metadata (indices,
dst_rel, tile structure) is shared by both layers.
"""

import sys

sys.path.insert(0, "/opt/trn_rl_repo")

import numpy as np
import ml_dtypes

import concourse.bass as bass
import concourse.mybir as mybir
from concourse import bacc
from concourse.tile import TileContext

N = 100000
E = 600000
IN_D = 128
HID_D = 128
OUT_D = 64
NCORES = 8
BLK = 128               # dst window size
NW = 98                 # windows per shard
SH = BLK * NW           # 12544 nodes per shard
NPADN = NCORES * SH     # 100352 padded node count
CHUNK = NPADN // 4      # gather chunk rows (25088; must be < 32768 for int16)
NCHUNK = 4              # ceil(NPADN / CHUNK)
BATCH_W = 14            # windows per gather batch
NBATCH = NW // BATCH_W  # 7
GCAP = 32               # max tiles (x128 idxs) per dma_gather call (needs single_packet=False)

f32 = mybir.dt.float32
bf16 = mybir.dt.bfloat16
i16 = mybir.dt.int16
bf16np = ml_dtypes.bfloat16

_cache = {}


def _preprocess(edge_index):
    """Build per-core padded message streams + the shared static structure."""
    src_all = np.concatenate([edge_index[0], np.arange(N, dtype=np.int64)])
    dst_all = np.concatenate([edge_index[1], np.arange(N, dtype=np.int64)])
    deg = np.bincount(dst_all, minlength=N).astype(np.float32)  # >= 1 (self loop)
    dinv = 1.0 / np.sqrt(deg)
    sdeg = np.sqrt(deg)

    # message streams: edges only (self-loops become per-window identity tiles)
    src_e = edge_index[0]
    dst_e = edge_index[1]
    HS = SH // 2  # half-shard rows
    r_all = src_e // SH
    row_all = src_e % SH
    posmap_src = np.where(row_all < HS, r_all * HS + row_all,
                          NCORES * HS + r_all * HS + (row_all - HS))
    core = dst_e // SH
    w_all = (dst_e % SH) // BLK
    k_all = posmap_src // CHUNK
    grp_all = w_all * NCHUNK + k_all  # 392 groups per core

    # group processing order: for b in batches, for k in chunks, for w in batch
    order_groups = []
    for b in range(NBATCH):
        for k in range(NCHUNK):
            for w in range(b * BATCH_W, (b + 1) * BATCH_W):
                order_groups.append(w * NCHUNK + k)
    order_groups = np.asarray(order_groups)
    grank = np.empty(NW * NCHUNK, np.int64)
    grank[order_groups] = np.arange(NW * NCHUNK)

    counts = np.zeros((NCORES, NW * NCHUNK), np.int64)
    per_core = []
    for c in range(NCORES):
        m = core == c
        s, g = posmap_src[m], grp_all[m]
        d = dst_e[m]
        counts[c] = np.bincount(g, minlength=NW * NCHUNK)
        per_core.append((s, d, g))

    T_wk = np.maximum((counts.max(axis=0) + BLK - 1) // BLK, 1)  # tiles per group
    # offsets in tile stream, in group processing order
    tiles_ord = T_wk[order_groups]
    tile_off_ord = np.concatenate([[0], np.cumsum(tiles_ord)])
    T_total = int(tile_off_ord[-1])
    tile_off = np.empty(NW * NCHUNK, np.int64)  # per group, in group-id space
    tile_off[order_groups] = tile_off_ord[:-1]
    L = T_total * BLK  # total message slots

    idx_arrs = np.zeros((NCORES, 128, L // 16), np.int16)
    dstrel_arrs = np.full((NCORES, 128, T_total), 300.0, np.float32)
    for c in range(NCORES):
        s, d, g = per_core[c]
        o = np.argsort(g, kind="stable")
        s, d, g = s[o], d[o], g[o]
        gstart = np.concatenate([[0], np.cumsum(np.bincount(g, minlength=NW * NCHUNK))])
        within = np.arange(len(g)) - gstart[g]
        pos = tile_off[g] * BLK + within
        rel = (s % CHUNK).astype(np.int16)
        idxw = np.zeros(L, np.int16)
        idxw[pos] = rel
        idxw = idxw.reshape(L // 16, 16).T  # [16, L/16]
        idx_arrs[c] = np.tile(idxw, (8, 1))
        drel = np.full(L, 300.0, np.float32)
        drel[pos] = (d % BLK).astype(np.float32)
        dstrel_arrs[c] = drel.reshape(T_total, 128).T  # [128, T]

    # static structure for codegen
    calls = []  # (b, k, ntiles, tile_start)
    for b in range(NBATCH):
        for k in range(NCHUNK):
            gs = [(b * BATCH_W + i) * NCHUNK + k for i in range(BATCH_W)]
            nt = int(T_wk[gs].sum())
            calls.append((b, k, nt, int(tile_off[gs[0]])))
    # per window: list of (chunk, column offset within that (b,k) call buffer)
    win_tiles = []
    for w in range(NW):
        b = w // BATCH_W
        lst = []
        for k in range(NCHUNK):
            gs0 = (b * BATCH_W) * NCHUNK + k
            coloff = int(tile_off[w * NCHUNK + k] - tile_off[gs0])
            for j in range(int(T_wk[w * NCHUNK + k])):
                lst.append((k, coloff + j))
        win_tiles.append(lst)

    g_ids = np.arange(NPADN)
    r_g = g_ids // SH
    row_g = g_ids % SH
    pos_g = np.where(row_g < HS, r_g * HS + row_g,
                     NCORES * HS + r_g * HS + (row_g - HS))
    return dict(
        pos_g=pos_g,
        deg=deg, dinv=dinv, sdeg=sdeg, T_total=T_total, L=L,
        idx_arrs=idx_arrs, dstrel_arrs=dstrel_arrs, calls=calls,
        win_tiles=win_tiles,
    )


def _build_nc(meta):
    T_total = meta["T_total"]
    L = meta["L"]
    calls = meta["calls"]
    win_tiles = meta["win_tiles"]

    nc = bacc.Bacc(None, target_bir_lowering=False, dynamic_dma_scratch_size=65536)

    xs_d = nc.dram_tensor("xs", [NPADN, IN_D], bf16, kind="ExternalInput")
    xso_d = nc.dram_tensor("xso", [SH, IN_D], bf16, kind="ExternalInput")
    pidx_d = nc.dram_tensor("pidx", [128, 1], f32, kind="ExternalInput")
    idx_d = nc.dram_tensor("idx", [128, L // 16], i16, kind="ExternalInput")
    dstrel_d = nc.dram_tensor("dstrel", [128, T_total], f32, kind="ExternalInput")
    iota_d = nc.dram_tensor("iota", [128, BLK], mybir.dt.float16, kind="ExternalInput")
    w1_d = nc.dram_tensor("w1", [IN_D, HID_D], bf16, kind="ExternalInput")
    w2_d = nc.dram_tensor("w2", [HID_D, OUT_D], bf16, kind="ExternalInput")
    b1_d = nc.dram_tensor("b1", [1, HID_D], f32, kind="ExternalInput")
    b2_d = nc.dram_tensor("b2", [1, OUT_D], f32, kind="ExternalInput")
    dinv_d = nc.dram_tensor("dinv", [128, NW], f32, kind="ExternalInput")
    dinv2_d = nc.dram_tensor("dinv2", [128, NW], f32, kind="ExternalInput")
    sdeg_d = nc.dram_tensor("sdeg", [1, SH], f32, kind="ExternalInput")
    out_d = nc.dram_tensor("out", [SH, OUT_D], f32, kind="ExternalOutput")

    HS = SH // 2
    h_send_a = nc.dram_tensor("h_send_a", [HS, HID_D], bf16)
    h_send_b = nc.dram_tensor("h_send_b", [HS, HID_D], bf16)
    h_full_a = nc.dram_tensor("h_full_a", [NCORES * HS, HID_D], bf16,
                              addr_space="Shared")
    h_full_b = nc.dram_tensor("h_full_b", [NCORES * HS, HID_D], bf16,
                              addr_space="Shared")

    with TileContext(nc) as tc:
        with (
            tc.tile_pool(name="const", bufs=1) as constp,
            tc.tile_pool(name="gath", bufs=2) as gathp,
            tc.tile_pool(name="sbuild", bufs=4) as sp,
            tc.tile_pool(name="agg", bufs=3) as aggp,
            tc.tile_pool(name="outp", bufs=3) as outp,
            tc.tile_pool(name="psum_seg", bufs=3, space="PSUM") as psegp,
            tc.tile_pool(name="psum_h", bufs=2, space="PSUM") as phk,
        ):
            # persistent tiles
            idx_t = constp.tile([128, L // 16], i16, tag="idx")
            dstrel_t = constp.tile([128, T_total], f32, tag="dstrel")
            iota_t = constp.tile([128, BLK], mybir.dt.float16, tag="iota")
            w1_t = constp.tile([IN_D, HID_D], bf16, tag="w1")
            w2_t = constp.tile([HID_D, OUT_D], bf16, tag="w2")
            b1_t = constp.tile([1, HID_D], f32, tag="b1")
            b2_t = constp.tile([1, OUT_D], f32, tag="b2")
            dinv_t = constp.tile([128, NW], f32, tag="dinv")
            dinv2_t = constp.tile([128, NW], f32, tag="dinv2")
            sdeg_t = constp.tile([1, SH], f32, tag="sdeg")
            pidx_t = constp.tile([128, 1], f32, tag="pidx")
            s_id = constp.tile([128, BLK], bf16, tag="s_id")
            nc.sync.dma_start(out=idx_t[:], in_=idx_d[:])
            nc.sync.dma_start(out=dstrel_t[:], in_=dstrel_d[:])
            nc.sync.dma_start(out=iota_t[:], in_=iota_d[:])
            nc.sync.dma_start(out=w1_t[:], in_=w1_d[:])
            nc.sync.dma_start(out=w2_t[:], in_=w2_d[:])
            nc.sync.dma_start(out=b1_t[:], in_=b1_d[:])
            nc.sync.dma_start(out=b2_t[:], in_=b2_d[:])
            nc.sync.dma_start(out=dinv_t[:], in_=dinv_d[:])
            nc.sync.dma_start(out=dinv2_t[:], in_=dinv2_d[:])
            nc.sync.dma_start(out=sdeg_t[:], in_=sdeg_d[:])
            nc.sync.dma_start(out=pidx_t[:], in_=pidx_d[:])
            nc.vector.tensor_scalar(
                s_id[:], iota_t[:], pidx_t[:, 0:1], None,
                mybir.AluOpType.is_equal,
            )

            def layer(tables, own_ap, w_t, b_t, od, out_dram, relu, out_dtype, scale_t, split_ab=False):
                for b in range(NBATCH):
                    bufs = {}
                    for (bb, k, nt, tstart) in calls:
                        if bb != b:
                            continue
                        g = gathp.tile([128, nt, IN_D], bf16, tag=f"g{k}")
                        for c0 in range(0, nt, GCAP):
                            ct = min(GCAP, nt - c0)
                            nidx = ct * BLK
                            nc.gpsimd.dma_gather(
                                g[:, c0:c0 + ct, :],
                                tables[k],
                                idx_t[:, (tstart + c0) * 8:
                                      (tstart + c0) * 8 + nidx // 16],
                                num_idxs=nidx, num_idxs_reg=nidx, elem_size=IN_D,
                                single_packet=False,
                            )
                        bufs[k] = g
                    for w in range(b * BATCH_W, (b + 1) * BATCH_W):
                        tl = win_tiles[w]

                        def seg_group(tl_part, tag, with_self):
                            pseg = psegp.tile([128, BLK], f32, tag=tag)
                            first = True
                            if with_self:
                                own = aggp.tile([128, IN_D], bf16, tag="own")
                                eng2 = nc.sync if (w % 2 == 1) else nc.scalar
                                if isinstance(own_ap, tuple):
                                    half = w // (NW // 2)
                                    wr = w % (NW // 2)
                                    eng2.dma_start(
                                        out=own[:],
                                        in_=own_ap[half][wr * BLK:(wr + 1) * BLK, :])
                                else:
                                    eng2.dma_start(
                                        out=own[:],
                                        in_=own_ap[w * BLK:(w + 1) * BLK, :])
                                nc.tensor.matmul(pseg[:], own[:], s_id[:],
                                                 start=True, stop=False)
                                first = False
                            for j, (k, col) in enumerate(tl_part):
                                s = sp.tile([128, BLK], bf16, tag="s")
                                tcol = None
                                for (bb, kk, nt2, ts2) in calls:
                                    if bb == b and kk == k:
                                        tcol = ts2 + col
                                        break
                                nc.vector.tensor_scalar(
                                    s[:], iota_t[:], dstrel_t[:, tcol:tcol + 1],
                                    None, mybir.AluOpType.is_equal,
                                )
                                nc.tensor.matmul(
                                    pseg[:], bufs[k][:, col, :], s[:],
                                    start=first,
                                    stop=(j == len(tl_part) - 1),
                                )
                                first = False
                            return pseg

                        ph = phk.tile([128, od], f32, tag="ph")
                        if split_ab:
                            tl_a = [t for t in tl if t[0] < 2]
                            tl_b = [t for t in tl if t[0] >= 2]
                            pseg_a = seg_group(tl_a, "psegA", True)
                            aggA = aggp.tile([128, BLK], bf16, tag="aggA")
                            nc.scalar.activation(
                                aggA[:], pseg_a[:], mybir.ActivationFunctionType.Copy)
                            nc.tensor.matmul(ph[:], aggA[:], w_t[:],
                                             start=True, stop=False)
                            pseg_b = seg_group(tl_b, "psegB", False)
                            aggB = aggp.tile([128, BLK], bf16, tag="aggB")
                            nc.scalar.activation(
                                aggB[:], pseg_b[:], mybir.ActivationFunctionType.Copy)
                            nc.tensor.matmul(ph[:], aggB[:], w_t[:],
                                             start=False, stop=False)
                        else:
                            pseg = seg_group(tl, "psegA", True)
                            aggT = aggp.tile([128, BLK], bf16, tag="aggA")
                            nc.scalar.activation(
                                aggT[:], pseg[:], mybir.ActivationFunctionType.Copy)
                            nc.tensor.matmul(ph[:], aggT[:], w_t[:],
                                             start=True, stop=False)
                        nc.tensor.matmul(
                            ph[:], sdeg_t[0:1, w * BLK:(w + 1) * BLK], b_t[:],
                            start=False, stop=True,
                        )
                        o = outp.tile([128, od], out_dtype, tag="o")
                        nc.scalar.activation(
                            o[:], ph[:],
                            mybir.ActivationFunctionType.Relu if relu
                            else mybir.ActivationFunctionType.Copy,
                            scale=scale_t[:, w:w + 1],
                        )
                        eng = nc.sync if (w % 2 == 0) else nc.scalar
                        if isinstance(out_dram, tuple):
                            half = w // (NW // 2)
                            wr = w % (NW // 2)
                            eng.dma_start(
                                out=out_dram[half][wr * BLK:(wr + 1) * BLK, :],
                                in_=o[:])
                        else:
                            eng.dma_start(
                                out=out_dram[w * BLK:(w + 1) * BLK, :], in_=o[:],
                            )

            def layer2(tables, own_ap, w_t, b_t, od, out_dram, scale_t):
                aggA = {}

                def sbuild(b, k, col):
                    s = sp.tile([128, BLK], bf16, tag="s")
                    tcol = None
                    for (bb, kk, nt2, ts2) in calls:
                        if bb == b and kk == k:
                            tcol = ts2 + col
                            break
                    nc.vector.tensor_scalar(
                        s[:], iota_t[:], dstrel_t[:, tcol:tcol + 1],
                        None, mybir.AluOpType.is_equal,
                    )
                    return s

                def gather_calls(b, ks):
                    bufs = {}
                    for (bb, k, nt, tstart) in calls:
                        if bb != b or k not in ks:
                            continue
                        g = gathp.tile([128, nt, IN_D], bf16, tag=f"g{k}")
                        for c0 in range(0, nt, GCAP):
                            ct = min(GCAP, nt - c0)
                            nidx = ct * BLK
                            nc.gpsimd.dma_gather(
                                g[:, c0:c0 + ct, :], tables[k],
                                idx_t[:, (tstart + c0) * 8:
                                      (tstart + c0) * 8 + nidx // 16],
                                num_idxs=nidx, num_idxs_reg=nidx,
                                elem_size=IN_D, single_packet=False,
                            )
                        bufs[k] = g
                    return bufs

                # pass A: chunks 0/1 + self tiles -> aggA (persistent SBUF)
                for b in range(NBATCH):
                    bufs = gather_calls(b, (0, 1))
                    for w in range(b * BATCH_W, (b + 1) * BATCH_W):
                        tl_a = [t for t in win_tiles[w] if t[0] < 2]
                        pseg = psegp.tile([128, BLK], f32, tag="psegA")
                        own = aggp.tile([128, IN_D], bf16, tag="own")
                        eng2 = nc.sync if (w % 2 == 1) else nc.scalar
                        half = w // (NW // 2)
                        wr = w % (NW // 2)
                        eng2.dma_start(
                            out=own[:],
                            in_=own_ap[half][wr * BLK:(wr + 1) * BLK, :])
                        nc.tensor.matmul(pseg[:], own[:], s_id[:],
                                         start=True, stop=False)
                        for j, (k, col) in enumerate(tl_a):
                            s = sbuild(b, k, col)
                            nc.tensor.matmul(
                                pseg[:], bufs[k][:, col, :], s[:],
                                start=False, stop=(j == len(tl_a) - 1),
                            )
                        ag = constp.tile([128, BLK], bf16, tag=f"aggA{w}")
                        nc.scalar.activation(
                            ag[:], pseg[:], mybir.ActivationFunctionType.Copy)
                        aggA[w] = ag
                # pass B: chunks 2/3, combine, transform, write out
                for b in range(NBATCH):
                    bufs = gather_calls(b, (2, 3))
                    for w in range(b * BATCH_W, (b + 1) * BATCH_W):
                        tl_b = [t for t in win_tiles[w] if t[0] >= 2]
                        pseg = psegp.tile([128, BLK], f32, tag="psegB")
                        for j, (k, col) in enumerate(tl_b):
                            s = sbuild(b, k, col)
                            nc.tensor.matmul(
                                pseg[:], bufs[k][:, col, :], s[:],
                                start=(j == 0), stop=(j == len(tl_b) - 1),
                            )
                        aggB = aggp.tile([128, BLK], bf16, tag="aggB")
                        nc.scalar.activation(
                            aggB[:], pseg[:], mybir.ActivationFunctionType.Copy)
                        ph = phk.tile([128, od], f32, tag="ph")
                        nc.tensor.matmul(ph[:], aggA[w][:], w_t[:],
                                         start=True, stop=False)
                        nc.tensor.matmul(ph[:], aggB[:], w_t[:],
                                         start=False, stop=False)
                        nc.tensor.matmul(
                            ph[:], sdeg_t[0:1, w * BLK:(w + 1) * BLK], b_t[:],
                            start=False, stop=True,
                        )
                        o = outp.tile([128, od], f32, tag="o")
                        nc.scalar.activation(
                            o[:], ph[:], mybir.ActivationFunctionType.Copy,
                            scale=scale_t[:, w:w + 1],
                        )
                        eng = nc.sync if (w % 2 == 0) else nc.scalar
                        eng.dma_start(
                            out=out_dram[w * BLK:(w + 1) * BLK, :], in_=o[:],
                        )

            xs_tables = [xs_d[k * CHUNK:(k + 1) * CHUNK, :] for k in range(NCHUNK)]
            layer(xs_tables, xso_d, w1_t, b1_t, HID_D, (h_send_a, h_send_b),
                  True, bf16, dinv2_t)
            if NCORES == 1:
                nc.sync.dma_start(out=h_full_a[:], in_=h_send_a[:])
                nc.sync.dma_start(out=h_full_b[:], in_=h_send_b[:])
            else:
                nc.gpsimd.collective_compute(
                    "AllGather", mybir.AluOpType.bypass,
                    replica_groups=[list(range(NCORES))],
                    ins=[h_send_a[:]], outs=[h_full_a[:]],
                )
                nc.gpsimd.collective_compute(
                    "AllGather", mybir.AluOpType.bypass,
                    replica_groups=[list(range(NCORES))],
                    ins=[h_send_b[:]], outs=[h_full_b[:]],
                )
            h_tables = [
                h_full_a[0:CHUNK, :], h_full_a[CHUNK:2 * CHUNK, :],
                h_full_b[0:CHUNK, :], h_full_b[CHUNK:2 * CHUNK, :],
            ]
            layer2(h_tables, (h_send_a, h_send_b), w2_t, b2_t, OUT_D, out_d,
                   dinv_t)

    nc.compile()
    return nc


def _get_runner(edge_index_bytes, edge_index):
    key = hash(edge_index_bytes)
    if key in _cache:
        return _cache[key]
    meta = _preprocess(edge_index.astype(np.int64))
    nc = _build_nc(meta)
    runner = _Runner(nc)
    _cache[key] = (meta, nc, runner)
    return _cache[key]


def _in_maps(meta, x, W1, b1, W2, b2):
    dinv = meta["dinv"]
    xs = (x * dinv[:, None]).astype(bf16np)
    xs = np.concatenate([xs, np.zeros((NPADN - N, IN_D), bf16np)], axis=0)
    xs_own_full = xs
    xs_r = np.empty_like(xs)
    xs_r[meta["pos_g"]] = xs
    xs = xs_r
    iota = np.broadcast_to(np.arange(BLK, dtype=np.float16), (128, BLK)).copy()
    dinv_p = np.concatenate([dinv, np.zeros(NPADN - N, np.float32)])
    sdeg_p = np.concatenate([meta["sdeg"], np.zeros(NPADN - N, np.float32)])
    maps = []
    for c in range(NCORES):
        dv = dinv_p[c * SH:(c + 1) * SH].reshape(NW, 128).T.copy()
        dv2 = (dv * dv).copy()
        sd = sdeg_p[c * SH:(c + 1) * SH].reshape(1, SH).copy()
        maps.append({
            "xs": xs,
            "xso": xs_own_full[c * SH:(c + 1) * SH],
            "pidx": np.arange(128, dtype=np.float32).reshape(128, 1),
            "idx": meta["idx_arrs"][c],
            "dstrel": meta["dstrel_arrs"][c],
            "iota": iota,
            "w1": W1.astype(bf16np),
            "w2": W2.astype(bf16np),
            "b1": b1.reshape(1, HID_D).astype(np.float32),
            "b2": b2.reshape(1, OUT_D).astype(np.float32),
            "dinv": dv,
            "dinv2": dv2,
            "sdeg": sd,
        })
    return maps


class _Runner:
    """Compile-once PJRT executor for a fixed Bass module (8-core SPMD)."""

    def __init__(self, nc):
        import jax
        from jax.sharding import Mesh, PartitionSpec
        from jax.experimental.shard_map import shard_map
        from concourse import bass2jax

        bass2jax.install_neuronx_cc_hook()
        self.nc = nc
        in_names, out_names, out_avals, zero_shapes = [], [], [], []
        pname = nc.partition_id_tensor.name if nc.partition_id_tensor else None
        for alloc in nc.m.functions[0].allocations:
            if not isinstance(alloc, mybir.MemoryLocationSet):
                continue
            name = alloc.memorylocations[0].name
            if alloc.kind == "ExternalInput":
                if name != pname:
                    in_names.append(name)
            elif alloc.kind == "ExternalOutput":
                out_names.append(name)
                shape = tuple(alloc.tensor_shape)
                dtype = mybir.dt.np(alloc.dtype)
                out_avals.append(jax.core.ShapedArray(shape, dtype))
                zero_shapes.append((shape, dtype))
        self.in_names, self.out_names = in_names, out_names
        self.zero_shapes = zero_shapes
        n_params, n_outs = len(in_names), len(out_names)
        all_names = in_names + out_names + ([pname] if pname else [])

        def _body(*args):
            operands = list(args)
            if pname is not None:
                operands.append(bass2jax.partition_id_tensor())
            outs = bass2jax._bass_exec_p.bind(
                *operands,
                out_avals=tuple(out_avals),
                in_names=tuple(all_names),
                out_names=tuple(out_names),
                lowering_input_output_aliases=(),
                sim_require_finite=True,
                sim_require_nnan=True,
                nc=nc,
            )
            return tuple(outs)

        devices = jax.devices()[:NCORES]
        mesh = Mesh(np.asarray(devices), ("core",))
        self.mesh = mesh
        in_specs = (PartitionSpec("core"),) * (n_params + n_outs)
        out_specs = (PartitionSpec("core"),) * n_outs
        self.fn = jax.jit(
            shard_map(_body, mesh=mesh, in_specs=in_specs, out_specs=out_specs,
                      check_rep=False),
            donate_argnums=tuple(range(n_params, n_params + n_outs)),
            keep_unused=True,
        )
        self.out_avals = out_avals

    def prep(self, in_maps):
        return [
            np.concatenate([np.asarray(in_maps[c][n]) for c in range(NCORES)],
                           axis=0)
            for n in self.in_names
        ]

    def zeros(self):
        return [np.zeros((NCORES * s[0], *s[1:]), d) for s, d in self.zero_shapes]

    def run_raw(self, concat_in, concat_zeros):
        import jax
        out_arrs = self.fn(*concat_in, *concat_zeros)
        jax.block_until_ready(out_arrs)
        return out_arrs

    def __call__(self, concat_in, concat_zeros):
        out_arrs = self.run_raw(concat_in, concat_zeros)
        return {
            n: np.asarray(out_arrs[i]).reshape(
                NCORES, *self.out_avals[i].shape)
            for i, n in enumerate(self.out_names)
        }


def kernel(x, edge_index, W1, b1, W2, b2):
    x = np.asarray(x, np.float32)
    edge_index = np.asarray(edge_index)
    meta, nc, runner = _get_runner(edge_index.tobytes(), edge_index)
    maps = _in_maps(meta, x, np.asarray(W1), np.asarray(b1), np.asarray(W2),
                    np.asarray(b2))
    res = runner(runner.prep(maps), runner.zeros())
    out = res["out"].reshape(NCORES * SH, OUT_D)
    return out[:N].astype(np.float32)



# revision 14
# speedup vs baseline: 2.7673x; 2.7673x over previous
"""GCN (2-layer) Trainium2 kernel, 8-core SPMD.

v2: the baseline was bound by dma_gather descriptor drain rate (~7ns/desc on
one SWDGE queue). This version:
  - spreads gathers across 4 SWDGE queues (measured ~1.5ns/desc aggregate),
  - folds self-loops into the gather stream (no per-window own-row DMAs),
  - shards the h exchange into 4 quarter AllGathers issued as soon as each
    quarter of layer 1 finishes, hiding them under compute,
  - batches h/out stores (one HWDGE DMA per quarter/batch).

Per dst-window (128 nodes) aggregation stays a PE segment-matmul
    aggT[feat, dst] += M[msgs, feat].T @ S[msgs, dst]
with M a dma_gather of dinv-prescaled source rows (bf16, 256B) and S a
one-hot built on DVE. Layer output: o = act(scale * (aggT.T @ W + sdeg*b)).

Node tables (xs for layer 1, h for layer 2) share one layout: 4 quarter
chunks; chunk q holds, for each core c, rows at position
c*qsize_q + p*nw_q + j  for node (core c, window qstart_q + j, partition p).
That makes the h quarter store one contiguous [128, nw_q*HID] DMA and the
AllGather output directly usable as the gather table.
"""

import sys

sys.path.insert(0, "/opt/trn_rl_repo")

import numpy as np
import ml_dtypes

import concourse.bass as bass
import concourse.mybir as mybir
from concourse import bacc
from concourse.tile import TileContext

N = 100000
E = 600000
IN_D = 128
HID_D = 128
OUT_D = 64
NCORES = 8
BLK = 128
NW = 98                  # windows per shard
SH = BLK * NW            # 12544 nodes per shard
NPADN = NCORES * SH      # 100352
NCHUNK = 4
QSTART = [0, 24, 49, 73, 98]          # window quarters
NWQ = [24, 25, 24, 25]                # windows per quarter
QROWS = [nw * BLK for nw in NWQ]      # rows per quarter per core
BATCH_W = 7
NBATCH = NW // BATCH_W   # 14
GCAP = 48                # max tiles per dma_gather call

f32 = mybir.dt.float32
bf16 = mybir.dt.bfloat16
i16 = mybir.dt.int16
f16 = mybir.dt.float16
bf16np = ml_dtypes.bfloat16

_cache = {}


def _node_pos(g):
    """Global node id -> (chunk, position within chunk table)."""
    c = g // SH
    r = g % SH
    w = r // BLK
    p = r % BLK
    q = np.searchsorted(np.asarray(QSTART), w, side="right") - 1
    j = w - np.asarray(QSTART)[q]
    pos = c * np.asarray(QROWS)[q] + p * np.asarray(NWQ)[q] + j
    return q, pos


def _preprocess(edge_index):
    src_e = edge_index[0]
    dst_e = edge_index[1]
    # self-loops as ordinary messages (all NPADN nodes; pad nodes are zero)
    loop = np.arange(NPADN, dtype=np.int64)
    src_all = np.concatenate([src_e, loop])
    dst_all = np.concatenate([dst_e, loop])

    deg = np.bincount(dst_e, minlength=NPADN).astype(np.float32) + 1.0
    deg[N:] = 0.0
    with np.errstate(divide="ignore"):
        dinv = np.where(deg > 0, 1.0 / np.sqrt(deg), 0.0).astype(np.float32)
    sdeg = np.sqrt(deg).astype(np.float32)

    k_all, pos_all = _node_pos(src_all)
    core = dst_all // SH
    w_all = (dst_all % SH) // BLK
    grp_all = w_all * NCHUNK + k_all

    counts = np.zeros((NCORES, NW * NCHUNK), np.int64)
    per_core = []
    for c in range(NCORES):
        m = core == c
        counts[c] = np.bincount(grp_all[m], minlength=NW * NCHUNK)
        per_core.append((pos_all[m], dst_all[m], grp_all[m]))

    T_wk = np.maximum((counts.max(axis=0) + BLK - 1) // BLK, 1)

    # group processing order: batches of 14 windows, chunk-major inside
    order_groups = []
    for b in range(NBATCH):
        for k in range(NCHUNK):
            for w in range(b * BATCH_W, (b + 1) * BATCH_W):
                order_groups.append(w * NCHUNK + k)
    order_groups = np.asarray(order_groups)
    tiles_ord = T_wk[order_groups]
    tile_off_ord = np.concatenate([[0], np.cumsum(tiles_ord)])
    T_total = int(tile_off_ord[-1])
    tile_off = np.empty(NW * NCHUNK, np.int64)
    tile_off[order_groups] = tile_off_ord[:-1]
    L = T_total * BLK

    # spread padding indices so consecutive pad descriptors don't hit one row
    idx_arrs = np.zeros((NCORES, 128, L // 16), np.int16)
    dstrel_arrs = np.full((NCORES, 128, T_total), 300.0, np.float32)
    chunk_rows = np.asarray(QROWS) * NCORES
    pad_base = (np.arange(L, dtype=np.int64) * 37)
    for c in range(NCORES):
        pos, d, g = per_core[c]
        o = np.argsort(g, kind="stable")
        pos, d, g = pos[o], d[o], g[o]
        gstart = np.concatenate([[0], np.cumsum(np.bincount(g, minlength=NW * NCHUNK))])
        within = np.arange(len(g)) - gstart[g]
        slot = tile_off[g] * BLK + within
        # pad indices: spread within the slot's own chunk
        tile_chunk = np.empty(T_total, np.int64)
        tile_chunk[:] = 0
        for gi in range(NW * NCHUNK):
            tile_chunk[tile_off[gi]:tile_off[gi] + T_wk[gi]] = gi % NCHUNK
        slot_chunk = np.repeat(tile_chunk, BLK)
        idxw = (pad_base % chunk_rows[slot_chunk]).astype(np.int16)
        idxw[slot] = pos.astype(np.int16)
        idxw = idxw.reshape(L // 16, 16).T
        idx_arrs[c] = np.tile(idxw, (8, 1))
        drel = np.full(L, 300.0, np.float32)
        drel[slot] = (d % BLK).astype(np.float32)
        dstrel_arrs[c] = drel.reshape(T_total, 128).T

    # per-(batch, chunk) call regions (in tile stream coordinates)
    regions = {}
    for b in range(NBATCH):
        for k in range(NCHUNK):
            gs = [(b * BATCH_W + i) * NCHUNK + k for i in range(BATCH_W)]
            regions[(b, k)] = (int(tile_off[gs[0]]), int(T_wk[gs].sum()))
    win_tiles = []
    for w in range(NW):
        b = w // BATCH_W
        lst = []
        for k in range(NCHUNK):
            gs0 = (b * BATCH_W) * NCHUNK + k
            coloff = int(tile_off[w * NCHUNK + k] - tile_off[gs0])
            for j in range(int(T_wk[w * NCHUNK + k])):
                lst.append((k, coloff + j))
        win_tiles.append(lst)

    return dict(
        deg=deg, dinv=dinv, sdeg=sdeg, T_total=T_total, L=L,
        idx_arrs=idx_arrs, dstrel_arrs=dstrel_arrs, regions=regions,
        win_tiles=win_tiles,
    )


def _build_nc(meta):
    T_total = meta["T_total"]
    L = meta["L"]
    regions = meta["regions"]
    win_tiles = meta["win_tiles"]

    nc = bacc.Bacc(None, target_bir_lowering=False,
                   dynamic_dma_scratch_size=65536,
                   num_swdge_queues=4)

    xs_d = [nc.dram_tensor(f"xs{q}", [NCORES * QROWS[q], IN_D], bf16,
                           kind="ExternalInput") for q in range(NCHUNK)]
    idx_d = nc.dram_tensor("idx", [128, L // 16], i16, kind="ExternalInput")
    dstrel_d = nc.dram_tensor("dstrel", [128, T_total], f32, kind="ExternalInput")
    iota_d = nc.dram_tensor("iota", [128, BLK], f16, kind="ExternalInput")
    w1_d = nc.dram_tensor("w1", [IN_D, HID_D], bf16, kind="ExternalInput")
    w2_d = nc.dram_tensor("w2", [HID_D, OUT_D], bf16, kind="ExternalInput")
    b1_d = nc.dram_tensor("b1", [1, HID_D], f32, kind="ExternalInput")
    b2_d = nc.dram_tensor("b2", [1, OUT_D], f32, kind="ExternalInput")
    dinv_d = nc.dram_tensor("dinv", [128, NW], f32, kind="ExternalInput")
    dinv2_d = nc.dram_tensor("dinv2", [128, NW], f32, kind="ExternalInput")
    sdeg_d = nc.dram_tensor("sdeg", [1, SH], f32, kind="ExternalInput")
    out_d = nc.dram_tensor("out", [128, NW * OUT_D], f32, kind="ExternalOutput")

    h_send = [nc.dram_tensor(f"h_send{q}", [128, NWQ[q] * HID_D], bf16)
              for q in range(NCHUNK)]
    h_full = [nc.dram_tensor(f"h_full{q}", [NCORES * 128, NWQ[q] * HID_D],
                             bf16, addr_space="Shared") for q in range(NCHUNK)]

    with TileContext(nc) as tc:
        with (
            tc.tile_pool(name="const", bufs=1) as constp,
            tc.tile_pool(name="gath", bufs=2) as gathp,
            tc.tile_pool(name="sbuild", bufs=4) as sp,
            tc.tile_pool(name="agg", bufs=3) as aggp,
            tc.tile_pool(name="stage", bufs=2) as stagep,
            tc.tile_pool(name="psum_seg", bufs=4, space="PSUM") as psegp,
            tc.tile_pool(name="psum_h", bufs=2, space="PSUM") as phk,
        ):
            idx_t = constp.tile([128, L // 16], i16, tag="idx")
            dstrel_t = constp.tile([128, T_total], f32, tag="dstrel")
            iota_t = constp.tile([128, BLK], f16, tag="iota")
            w1_t = constp.tile([IN_D, HID_D], bf16, tag="w1")
            w2_t = constp.tile([HID_D, OUT_D], bf16, tag="w2")
            b1_t = constp.tile([1, HID_D], f32, tag="b1")
            b2_t = constp.tile([1, OUT_D], f32, tag="b2")
            dinv_t = constp.tile([128, NW], f32, tag="dinv")
            dinv2_t = constp.tile([128, NW], f32, tag="dinv2")
            sdeg_t = constp.tile([1, SH], f32, tag="sdeg")
            nc.sync.dma_start(out=idx_t[:], in_=idx_d[:])
            nc.sync.dma_start(out=dstrel_t[:], in_=dstrel_d[:])
            nc.sync.dma_start(out=iota_t[:], in_=iota_d[:])
            nc.sync.dma_start(out=w1_t[:], in_=w1_d[:])
            nc.sync.dma_start(out=w2_t[:], in_=w2_d[:])
            nc.sync.dma_start(out=b1_t[:], in_=b1_d[:])
            nc.sync.dma_start(out=b2_t[:], in_=b2_d[:])
            nc.sync.dma_start(out=dinv_t[:], in_=dinv_d[:])
            nc.sync.dma_start(out=dinv2_t[:], in_=dinv2_d[:])
            nc.sync.dma_start(out=sdeg_t[:], in_=sdeg_d[:])

            def sbuild(tcol):
                s = sp.tile([128, BLK], bf16, tag="s")
                nc.vector.tensor_scalar(
                    s[:], iota_t[:], dstrel_t[:, tcol:tcol + 1],
                    None, mybir.AluOpType.is_equal,
                )
                return s

            def gather_calls(tables, b, ks):
                """Issue gathers for chunks `ks` of batch b; each chunk is
                split into 2 calls (own pool tile each). queue_num is
                rewritten after scheduling to match the DMASW lane."""
                bufs = {}
                for k in ks:
                    tstart, nt = regions[(b, k)]
                    assert nt <= 2 * GCAP
                    splits = [0, nt // 2, nt]
                    pieces = []
                    for i in range(2):
                        c0, c1 = splits[i], splits[i + 1]
                        if c1 <= c0:
                            continue
                        g = gathp.tile([128, c1 - c0, IN_D], bf16,
                                       tag=f"g{k}_{i}")
                        nidx = (c1 - c0) * BLK
                        nc.gpsimd.dma_gather(
                            g[:], tables[k],
                            idx_t[:, (tstart + c0) * 8:
                                  (tstart + c0) * 8 + nidx // 16],
                            num_idxs=nidx, num_idxs_reg=nidx,
                            elem_size=IN_D, single_packet=False,
                            queue_num=0,
                        )
                        pieces.append((c0, c1, g))
                    bufs[k] = pieces
                return bufs

            def seg_accum(bufs, tl, tag):
                pseg = psegp.tile([128, BLK], f32, tag=tag)
                for j, (k, col) in enumerate(tl):
                    s = sbuild(col_abs(k, col))
                    g = None
                    for (c0, c1, gg) in bufs[k]:
                        if c0 <= col < c1:
                            g = gg[:, col - c0, :]
                            break
                    nc.tensor.matmul(
                        pseg[:], g, s[:],
                        start=(j == 0), stop=(j == len(tl) - 1),
                    )
                return pseg

            cur_b = [0]

            def col_abs(k, col):
                tstart, _ = regions[(cur_b[0], k)]
                return tstart + col

            # ---------------- layer 1 (single pass) ----------------
            xs_tables = [t[:] for t in xs_d]
            stage_h = None
            for b in range(NBATCH):
                cur_b[0] = b
                bufs = gather_calls(xs_tables, b, (0, 1, 2, 3))
                for w in range(b * BATCH_W, (b + 1) * BATCH_W):
                    q = next(i for i in range(4)
                             if QSTART[i] <= w < QSTART[i + 1])
                    j = w - QSTART[q]
                    if j == 0:
                        stage_h = stagep.tile([128, 25, HID_D], bf16,
                                              tag="stage_h")
                    pseg = seg_accum(bufs, win_tiles[w], "pseg")
                    aggT = aggp.tile([128, BLK], bf16, tag="aggT")
                    nc.scalar.activation(
                        aggT[:], pseg[:], mybir.ActivationFunctionType.Copy)
                    ph = phk.tile([128, HID_D], f32, tag="ph")
                    nc.tensor.matmul(ph[:], aggT[:], w1_t[:],
                                     start=True, stop=False)
                    nc.tensor.matmul(
                        ph[:], sdeg_t[0:1, w * BLK:(w + 1) * BLK], b1_t[:],
                        start=False, stop=True,
                    )
                    nc.scalar.activation(
                        stage_h[:, j, :], ph[:],
                        mybir.ActivationFunctionType.Relu,
                        scale=dinv2_t[:, w:w + 1],
                    )
                    if j == NWQ[q] - 1:
                        eng = nc.sync if (q % 2 == 0) else nc.scalar
                        eng.dma_start(
                            out=h_send[q][:],
                            in_=stage_h[:, 0:NWQ[q], :].rearrange(
                                "a b c -> a (b c)"),
                        )
                        nc.gpsimd.collective_compute(
                            "AllGather", mybir.AluOpType.bypass,
                            replica_groups=[list(range(NCORES))],
                            ins=[h_send[q][:]], outs=[h_full[q][:]],
                        )

            h_tables = [
                h_full[q][:].rearrange("a (b d) -> (a b) d", d=HID_D)
                for q in range(NCHUNK)
            ]

            # ---------------- layer 2 pass A: chunks 0/1 ----------------
            aggA = {}
            for b in range(NBATCH):
                cur_b[0] = b
                bufs = gather_calls(h_tables, b, (0, 1))
                for w in range(b * BATCH_W, (b + 1) * BATCH_W):
                    tl = [t for t in win_tiles[w] if t[0] < 2]
                    pseg = seg_accum(bufs, tl, "pseg")
                    ag = constp.tile([128, BLK], bf16, tag=f"aggA{w}")
                    nc.scalar.activation(
                        ag[:], pseg[:], mybir.ActivationFunctionType.Copy)
                    aggA[w] = ag

            # ---------------- layer 2 pass B: chunks 2/3 + combine ------
            stage_o = None
            for b in range(NBATCH):
                cur_b[0] = b
                bufs = gather_calls(h_tables, b, (2, 3))
                stage_o = stagep.tile([128, BATCH_W, OUT_D], f32, tag="stage_o")
                for w in range(b * BATCH_W, (b + 1) * BATCH_W):
                    tl = [t for t in win_tiles[w] if t[0] >= 2]
                    pseg = seg_accum(bufs, tl, "pseg")
                    aggB = aggp.tile([128, BLK], bf16, tag="aggB")
                    nc.scalar.activation(
                        aggB[:], pseg[:], mybir.ActivationFunctionType.Copy)
                    ph = phk.tile([128, OUT_D], f32, tag="ph")
                    nc.tensor.matmul(ph[:], aggA[w][:], w2_t[:],
                                     start=True, stop=False)
                    nc.tensor.matmul(ph[:], aggB[:], w2_t[:],
                                     start=False, stop=False)
                    nc.tensor.matmul(
                        ph[:], sdeg_t[0:1, w * BLK:(w + 1) * BLK], b2_t[:],
                        start=False, stop=True,
                    )
                    nc.scalar.activation(
                        stage_o[:, w - b * BATCH_W, :], ph[:],
                        mybir.ActivationFunctionType.Copy,
                        scale=dinv_t[:, w:w + 1],
                    )
                eng = nc.sync if (b % 2 == 0) else nc.scalar
                eng.dma_start(
                    out=out_d[:, b * BATCH_W * OUT_D:(b + 1) * BATCH_W * OUT_D],
                    in_=stage_o[:].rearrange("a b c -> a (b c)"),
                )

    # Tile assigns DMASW completion sems round-robin (8 lanes) over
    # Pool-engine DMA instructions in scheduled order, ignoring queue_num.
    # Each sem must be used by exactly one SWDGE queue, so derive queue_num
    # from the assigned lane: queue = lane % 4 (a fixed lane->queue map).
    import re as _re
    for inst in nc.inst_map.values():
        if isinstance(inst, mybir.InstDMAGatherAnt):
            lane = None
            si = inst.sync_info
            for u in (si.on_update if si else []):
                m = _re.match(r"DMASW(\d+)_", u.ant_name or "")
                if m:
                    lane = int(m.group(1))
                    break
            assert lane is not None, f"no DMASW lane on {inst.name}"
            inst.queue_num = lane % 4

    nc.compile()
    return nc


def _get_runner(edge_index_bytes, edge_index):
    key = hash(edge_index_bytes)
    if key in _cache:
        return _cache[key]
    meta = _preprocess(edge_index.astype(np.int64))
    nc = _build_nc(meta)
    runner = _Runner(nc)
    _cache[key] = (meta, nc, runner)
    return _cache[key]


def _in_maps(meta, x, W1, b1, W2, b2):
    dinv = meta["dinv"]
    xs = (x * dinv[:N, None]).astype(bf16np)
    xs = np.concatenate([xs, np.zeros((NPADN - N, IN_D), bf16np)], axis=0)
    # reorder into quarter-chunk table layout
    g_ids = np.arange(NPADN)
    q_g, pos_g = _node_pos(g_ids)
    xs_chunks = [np.zeros((NCORES * QROWS[q], IN_D), bf16np)
                 for q in range(NCHUNK)]
    for q in range(NCHUNK):
        m = q_g == q
        xs_chunks[q][pos_g[m]] = xs[m]
    iota = np.broadcast_to(np.arange(BLK, dtype=np.float16), (128, BLK)).copy()
    dinv_p = dinv
    sdeg_p = meta["sdeg"]
    maps = []
    for c in range(NCORES):
        dv = dinv_p[c * SH:(c + 1) * SH].reshape(NW, 128).T.copy()
        dv2 = (dv * dv).copy()
        sd = sdeg_p[c * SH:(c + 1) * SH].reshape(1, SH).copy()
        mp = {
            "idx": meta["idx_arrs"][c],
            "dstrel": meta["dstrel_arrs"][c],
            "iota": iota,
            "w1": np.asarray(W1).astype(bf16np),
            "w2": np.asarray(W2).astype(bf16np),
            "b1": np.asarray(b1).reshape(1, HID_D).astype(np.float32),
            "b2": np.asarray(b2).reshape(1, OUT_D).astype(np.float32),
            "dinv": dv,
            "dinv2": dv2,
            "sdeg": sd,
        }
        for q in range(NCHUNK):
            mp[f"xs{q}"] = xs_chunks[q]
        maps.append(mp)
    return maps


class _Runner:
    """Compile-once PJRT executor for a fixed Bass module (8-core SPMD)."""

    def __init__(self, nc):
        import jax
        from jax.sharding import Mesh, PartitionSpec
        from jax.experimental.shard_map import shard_map
        from concourse import bass2jax

        bass2jax.install_neuronx_cc_hook()
        self.nc = nc
        in_names, out_names, out_avals, zero_shapes = [], [], [], []
        pname = nc.partition_id_tensor.name if nc.partition_id_tensor else None
        for alloc in nc.m.functions[0].allocations:
            if not isinstance(alloc, mybir.MemoryLocationSet):
                continue
            name = alloc.memorylocations[0].name
            if alloc.kind == "ExternalInput":
                if name != pname:
                    in_names.append(name)
            elif alloc.kind == "ExternalOutput":
                out_names.append(name)
                shape = tuple(alloc.tensor_shape)
                dtype = mybir.dt.np(alloc.dtype)
                out_avals.append(jax.core.ShapedArray(shape, dtype))
                zero_shapes.append((shape, dtype))
        self.in_names, self.out_names = in_names, out_names
        self.zero_shapes = zero_shapes
        n_params, n_outs = len(in_names), len(out_names)
        all_names = in_names + out_names + ([pname] if pname else [])

        def _body(*args):
            operands = list(args)
            if pname is not None:
                operands.append(bass2jax.partition_id_tensor())
            outs = bass2jax._bass_exec_p.bind(
                *operands,
                out_avals=tuple(out_avals),
                in_names=tuple(all_names),
                out_names=tuple(out_names),
                lowering_input_output_aliases=(),
                sim_require_finite=True,
                sim_require_nnan=True,
                nc=nc,
            )
            return tuple(outs)

        devices = jax.devices()[:NCORES]
        mesh = Mesh(np.asarray(devices), ("core",))
        self.mesh = mesh
        in_specs = (PartitionSpec("core"),) * (n_params + n_outs)
        out_specs = (PartitionSpec("core"),) * n_outs
        self.fn = jax.jit(
            shard_map(_body, mesh=mesh, in_specs=in_specs, out_specs=out_specs,
                      check_rep=False),
            donate_argnums=tuple(range(n_params, n_params + n_outs)),
            keep_unused=True,
        )
        self.out_avals = out_avals

    def prep(self, in_maps):
        return [
            np.concatenate([np.asarray(in_maps[c][n]) for c in range(NCORES)],
                           axis=0)
            for n in self.in_names
        ]

    def zeros(self):
        return [np.zeros((NCORES * s[0], *s[1:]), d) for s, d in self.zero_shapes]

    def run_raw(self, concat_in, concat_zeros):
        import jax
        out_arrs = self.fn(*concat_in, *concat_zeros)
        jax.block_until_ready(out_arrs)
        return out_arrs

    def __call__(self, concat_in, concat_zeros):
        out_arrs = self.run_raw(concat_in, concat_zeros)
        return {
            n: np.asarray(out_arrs[i]).reshape(
                NCORES, *self.out_avals[i].shape)
            for i, n in enumerate(self.out_names)
        }


def kernel(x, edge_index, W1, b1, W2, b2):
    x = np.asarray(x, np.float32)
    edge_index = np.asarray(edge_index)
    meta, nc, runner = _get_runner(edge_index.tobytes(), edge_index)
    maps = _in_maps(meta, x, np.asarray(W1), np.asarray(b1), np.asarray(W2),
                    np.asarray(b2))
    res = runner(runner.prep(maps), runner.zeros())
    # out layout: [core, 128 part, NW*OUT_D] -> node (c, w*128+p)
    out = res["out"].reshape(NCORES, 128, NW, OUT_D).transpose(0, 2, 1, 3)
    out = out.reshape(NCORES * SH, OUT_D)
    return out[:N].astype(np.float32)
